# revision 1
# baseline (speedup 1.0000x reference)
"""LIF (leaky integrate-and-fire) spiking recurrence on 8 Trainium2 cores.

Full input x: [T*bs, C, H, W] = [256, 128, 32, 32] f32 with T=8, bs=32.
Recurrence over T only, elementwise elsewhere:
    u_t = TAU * u_{t-1} * (1 - (u_{t-1} > VTH)) + x_t ;  o_t = (u_t > VTH)

Sharding: fully data-parallel over batch (bs=32 -> 4 per core), no collectives.

Since the op is elementwise outside of T, each core views its [4,128,32,32]
per-timestep slab as a flat [128 partitions, 4096] tile (16 KiB contiguous
HBM run per partition -> large DMA descriptors). Each timestep is split into
CH chunks so compute and output stores start as early as possible; the two
chunk chains interleave on DVE and hide cross-engine stalls.

Per step and chunk:
  DVE : u = (p * TAU) + x_t            (scalar_tensor_tensor; t=0: u=x_0)
  ACT : s = sign(VTH - u); o = relu(-s) = (u > VTH)   (exact: u-VTH exact near VTH)
  DVE : p = (u <= VTH) * u             (skipped at t=T-1)
TAU=0.5 is a power of two and the masks are 0/1, so everything except the
add is exact -> bitwise identical to the f32 reference.
"""

import numpy as np

import concourse.tile as tile
from concourse import bacc, mybir
from concourse.bass_utils import run_bass_kernel_spmd

T = 8
BS = 32
C = 128
HW = 32 * 32
NCORES = 8
BSH = BS // NCORES          # 4 batch elements per core
P = 128                     # SBUF partitions
FREE = BSH * C * HW // P    # 4096 f32 per partition per timestep
CH = 2                      # chunks per timestep
CHF = FREE // CH            # 2048
VTH = 1.0
TAU = 0.5
F32 = mybir.dt.float32

_nc_cache = None


def _build():
    nc = bacc.Bacc("TRN2", target_bir_lowering=False, debug=False, num_devices=NCORES)
    x_d = nc.dram_tensor("x", [T, P, FREE], F32, kind="ExternalInput").ap()
    o_d = nc.dram_tensor("o", [T, P, FREE], F32, kind="ExternalOutput").ap()

    BF16 = mybir.dt.bfloat16

    with tile.TileContext(nc) as tc:
        with (
            tc.tile_pool(name="xa", bufs=1) as xa,
            tc.tile_pool(name="pp", bufs=1) as pp,
            tc.tile_pool(name="sp", bufs=2) as sp,
            tc.tile_pool(name="op", bufs=2) as op,
        ):
            # The whole 16 MiB per-core input stays resident in SBUF
            # (128 KiB/partition); the u state is computed in place over it,
            # so no separate u pool is needed. Subtile dependency tracking
            # lets each compute chunk start as soon as the load covering its
            # range lands.
            xt = xa.tile([P, T * FREE], F32)
            xv = x_d.rearrange("t p f -> p t f")  # [128, T, FREE] HBM view

            # Ramped load sizes (units of CHF=2048 halves): small first so
            # compute starts early, large later so the SP ring issues only a
            # few DMAs and the full input is resident early.
            load_ranges = [(0, 1), (1, 2), (2, 4), (4, 6), (6, 8), (8, 12), (12, 16)]
            for a, b in load_ranges:
                t0, f0 = divmod(a * CHF, FREE)
                t1, f1 = divmod(b * CHF, FREE)
                if f0 == 0 and f1 == 0:
                    src = xv[:, t0:t1, :]
                else:
                    assert t1 == t0 and f1 > f0 or (t1 == t0 + 1 and f1 == 0)
                    src = xv[:, t0, f0:f1 if f1 else FREE]
                nc.sync.dma_start(out=xt[:, a * CHF:b * CHF], in_=src)

            p_prev = None
            for t in range(T):
                # Full-timestep ops in the steady state (fewest instructions
                # and semaphores); halves at t=0 for an early pipeline start
                # and at t=T-1 for a short latency tail.
                nh = 2 if t in (0, T - 1) else 1
                w = FREE // nh
                s = sp.tile([P, FREE], BF16, name="s", tag="s")
                o = op.tile([P, FREE], F32, name="o", tag="o")
                pn = (
                    pp.tile([P, FREE], F32, name="p", tag="p")
                    if t < T - 1 else None
                )
                for c in range(nh):
                    fsl = slice(c * w, (c + 1) * w)
                    xsl = xt[:, t * FREE + c * w:t * FREE + (c + 1) * w]
                    if t > 0:
                        # u = p*TAU + x, in place over the x slice
                        nc.vector.scalar_tensor_tensor(
                            xsl, p_prev[:, fsl], TAU, xsl,
                            op0=mybir.AluOpType.mult, op1=mybir.AluOpType.add,
                        )
                    # s = sign(VTH - u) in bf16 (exact on {-1,0,1}), then
                    # o = relu(-s) = (u > VTH) in f32 for the store. Signs
                    # are flipped via the scale immediate because only
                    # 0.0/1.0 have pre-registered const APs for the bias.
                    nc.scalar.activation(
                        s[:, fsl], xsl, mybir.ActivationFunctionType.Sign,
                        bias=VTH, scale=-1.0,
                    )
                    nc.scalar.activation(
                        o[:, fsl], s[:, fsl],
                        mybir.ActivationFunctionType.Relu, scale=-1.0,
                    )
                    if pn is not None:
                        nc.vector.scalar_tensor_tensor(
                            pn[:, fsl], xsl, VTH, xsl,
                            op0=mybir.AluOpType.is_le, op1=mybir.AluOpType.mult,
                        )
                    # Stores go out on the GpSimd SWDGE ring: its queue rows
                    # are separate from the SP HWDGE ring, so stores are not
                    # FIFO-blocked behind the big prefetch loads. The last
                    # timestep's stores are quartered for a short tail.
                    if t == T - 1:
                        q = w // 2
                        nc.gpsimd.dma_start(
                            out=o_d[t][:, c * w:c * w + q],
                            in_=o[:, c * w:c * w + q],
                        )
                        nc.gpsimd.dma_start(
                            out=o_d[t][:, c * w + q:(c + 1) * w],
                            in_=o[:, c * w + q:(c + 1) * w],
                        )
                    else:
                        nc.gpsimd.dma_start(out=o_d[t][:, fsl], in_=o[:, fsl])
                p_prev = pn

    nc.compile()
    return nc


def _get_nc():
    global _nc_cache
    if _nc_cache is None:
        _nc_cache = _build()
    return _nc_cache


def _run(x: np.ndarray, **spmd_kwargs):
    nc = _get_nc()
    xr = np.ascontiguousarray(np.asarray(x, dtype=np.float32)).reshape(T, BS, C, HW)
    in_maps = [
        {"x": np.ascontiguousarray(xr[:, k * BSH:(k + 1) * BSH]).reshape(T, P, FREE)}
        for k in range(NCORES)
    ]
    res = run_bass_kernel_spmd(nc, in_maps, core_ids=list(range(NCORES)), **spmd_kwargs)
    out = np.empty((T, BS, C, HW), dtype=np.float32)
    for k in range(NCORES):
        out[:, k * BSH:(k + 1) * BSH] = res.results[k]["o"].reshape(T, BSH, C, HW)
    return out.reshape(T * BS, C, 32, 32), res


def kernel(x: np.ndarray) -> np.ndarray:
    out, _ = _run(x)
    return out



# revision 8
# speedup vs baseline: 1.4463x; 1.4463x over previous
"""LIF (leaky integrate-and-fire) spiking recurrence on 8 Trainium2 cores.

Full input x: [T*bs, C, H, W] = [256, 128, 32, 32] f32 with T=8, bs=32.
Recurrence over T only, elementwise elsewhere:
    u_t = TAU * u_{t-1} * (1 - (u_{t-1} > VTH)) + x_t ;  o_t = (u_t > VTH)

Sharding: fully data-parallel over batch (bs=32 -> 4 per core), no collectives.

Each core views its per-timestep [4,128,32,32] slab as a [128, 4096] tile.
The kernel is HBM-bound on the 16 MiB input; the output (pure 0/1 spikes)
is bit-packed on device so stores are 1 MiB instead of 16 MiB:

  DVE : one fused custom op per step   u_t = x_t + TAU * u_{t-1} * (u_{t-1} <= VTH)
        (in-place over the x_t slab; bitwise identical to f32 reference)
  ACT : s_t = Sign(VTH - u_t) in bf16  (-1 = spike, +1 = no spike)
  PE  : psum += diag(-2^(t-1)) @ s_t   (accumulates packed = sum_t 2^t*o_t - 127.5)
  ACT : packed_bf16 = psum + 127.5     (integers 0..255, exact in bf16)
  DMA : store packed [128, 4096] bf16; host unpacks bits to the f32 output.

TAU=0.5 is a power of two, masks are 0/1, and the per-step add matches the
reference's rounding, so u_t is bit-exact; spikes differ only on the
measure-zero event u_t == VTH exactly.
"""

import numpy as np
import ml_dtypes

import concourse.tile as tile
from concourse import bacc, mybir
from concourse.bass_utils import run_bass_kernel_spmd

T = 8
BS = 32
C = 128
HW = 32 * 32
NCORES = 8
BSH = BS // NCORES          # 4 batch elements per core
P = 128                     # SBUF partitions
FREE = BSH * C * HW // P    # 4096 f32 per partition per timestep
VTH = 1.0
TAU = 0.5
F32 = mybir.dt.float32
BF16 = mybir.dt.bfloat16

_nc_cache = None
_lif_op_cache = None


def _register_lif_op():
    """Register the fused LIF-step custom DVE op:
        out = Src1 + (Src0 * C0) * (Src0 <= One)
    i.e. u_new = x + TAU * u * (u <= VTH), one DVE pass, 2 tensor reads."""
    global _lif_op_cache
    if _lif_op_cache is not None:
        return _lif_op_cache
    import concourse.dve_ops as dve_ops
    from concourse.dve_spec import Spec, Src0, Src1, C0, One, lower
    from concourse.dve_uop import DveOpSpec

    name = "LIF_STEP_ANT"
    for op in dve_ops.OPS:
        if op.name == name:
            _lif_op_cache = op
            return op

    spec = Spec(
        body=Src1 + (Src0 * C0) * (Src0 <= One),
        reference=lambda in0, in1, c0, c1, c2: in1
        + (in0 * np.float32(c0)) * (in0 <= np.float32(1.0)),
    )
    row = dve_ops._CUSTOM_DVE_ROW_BASE + len(dve_ops.OPS)
    uops_sha = {}
    for ver in ("v3", "v4"):
        try:
            s = DveOpSpec(
                name=name, opcode=row, uops=lower(spec, ver=ver), rd1_en=True
            )
            uops_sha[ver] = s.sha(ver)
        except Exception:
            pass
    op = dve_ops.DveOp(name, spec, subdim=False, uops_sha=uops_sha)
    dve_ops.OPS.append(op)
    dve_ops._SUB_OPCODE_FOR_NAME[name] = row
    dve_ops.CUSTOM_DVE_SPECS[name] = spec
    _lif_op_cache = op
    return op


def _chunks(t):
    # Column chunking per timestep: small chunks at the pipeline head (t=0,
    # before the DVE chain starts) and tail (t=T-1, shortens the critical
    # path after the last load lands); full halves in the steady state.
    if t == 0:
        return [(0, 1024), (1024, 2048), (2048, 4096)]
    if t == T - 1:
        return [(0, 2048), (2048, 3072), (3072, 4096)]
    return [(0, 2048), (2048, 4096)]


def _build():
    lif_op = _register_lif_op()
    nc = bacc.Bacc("TRN2", target_bir_lowering=False, debug=False, num_devices=NCORES)
    x_d = nc.dram_tensor("x", [T, P, FREE], F32, kind="ExternalInput").ap()
    w_d = nc.dram_tensor("w", [P, T * 128], BF16, kind="ExternalInput").ap()
    o_d = nc.dram_tensor("o_pk", [P, FREE], BF16, kind="ExternalOutput").ap()

    with tile.TileContext(nc) as tc:
        with (
            tc.tile_pool(name="xa", bufs=1) as xa,
            tc.tile_pool(name="wp", bufs=1) as wp,
            tc.tile_pool(name="sp", bufs=3) as sp,
            tc.tile_pool(name="pk", bufs=1) as pkp,
            tc.tile_pool(name="cb", bufs=1) as cb,
            tc.tile_pool(name="ps", bufs=1, space="PSUM") as ps,
        ):
            # Whole 16 MiB per-core input resident in SBUF (128 KiB/partition);
            # u_t is computed in place over the x_t slab. Subtile dependency
            # tracking lets each compute chunk start once its load lands.
            xt = xa.tile([P, T * FREE], F32)
            xv = x_d.rearrange("t p f -> p t f")  # [128, T, FREE] HBM view

            wsb = wp.tile([P, T * 128], BF16)     # 8 stationary diag matrices
            nc.gpsimd.dma_start(out=wsb[:, :], in_=w_d)

            bias = cb.tile([P, 1], F32)
            nc.vector.memset(bias[:, :], 127.5)

            # Ramped load sizes (units of 2048 cols = 1 MiB): small first so
            # compute starts early, large later for few DMAs at line rate.
            load_ranges = [(0, 1), (1, 2), (2, 4), (4, 6), (6, 8), (8, 12), (12, 16)]
            for a, b in load_ranges:
                t0, f0 = divmod(a * 2048, FREE)
                t1, f1 = divmod(b * 2048, FREE)
                if f0 == 0 and f1 == 0:
                    src = xv[:, t0:t1, :]
                else:
                    src = xv[:, t0, f0:f1 if f1 else FREE]
                nc.sync.dma_start(out=xt[:, a * 2048:b * 2048], in_=src)

            psum = ps.tile([P, FREE], F32)        # packed-spike accumulator
            pk = pkp.tile([P, FREE], BF16)

            for t in range(T):
                s = sp.tile([P, FREE], BF16, name="s", tag="s")
                for a, b in _chunks(t):
                    xsl = xt[:, t * FREE + a:t * FREE + b]
                    if t > 0:
                        usl = xt[:, (t - 1) * FREE + a:(t - 1) * FREE + b]
                        nc.vector._custom_dve(
                            lif_op, out=xsl, in0=usl, in1=xsl, s0=TAU
                        )
                    # s = sign(VTH - u) in bf16: -1 where spiking, +1 where not.
                    nc.scalar.activation(
                        s[:, a:b], xsl, mybir.ActivationFunctionType.Sign,
                        bias=VTH, scale=-1.0,
                    )
                    # psum += diag(-2^(t-1)) @ s  accumulated over t; after
                    # t=T-1: psum = sum_t 2^t * o_t - 127.5.
                    for blk in range(a, b, 512):
                        nc.tensor.matmul(
                            psum[:, blk:blk + 512],
                            wsb[:, t * 128:(t + 1) * 128],
                            s[:, blk:blk + 512],
                            start=(t == 0),
                            stop=(t == T - 1),
                        )
            # Convert psum -> packed bytes (as bf16: 0..255 ints are exact),
            # chunked so stores start while the last matmuls still run.
            for a, b in [(0, 1024), (1024, 2048), (2048, 3072), (3072, 4096)]:
                nc.scalar.activation(
                    pk[:, a:b], psum[:, a:b],
                    mybir.ActivationFunctionType.Identity,
                    bias=bias[:, :], scale=1.0,
                )
                nc.gpsimd.dma_start(out=o_d[:, a:b], in_=pk[:, a:b])

    nc.compile()
    return nc


def _get_nc():
    global _nc_cache
    if _nc_cache is None:
        _nc_cache = _build()
    return _nc_cache


def _make_w():
    w = np.zeros((T, 128, 128), np.float32)
    for t in range(T):
        np.fill_diagonal(w[t], -(2.0 ** (t - 1)))
    # SBUF layout: [partition k, t, m] -> [128, T*128]
    return np.ascontiguousarray(w.transpose(1, 0, 2)).reshape(P, T * 128).astype(
        ml_dtypes.bfloat16
    )


def _run(x: np.ndarray, **spmd_kwargs):
    nc = _get_nc()
    xr = np.ascontiguousarray(np.asarray(x, dtype=np.float32)).reshape(T, BS, C, HW)
    wl = _make_w()
    in_maps = [
        {
            "x": np.ascontiguousarray(xr[:, k * BSH:(k + 1) * BSH]).reshape(T, P, FREE),
            "w": wl,
        }
        for k in range(NCORES)
    ]
    res = run_bass_kernel_spmd(nc, in_maps, core_ids=list(range(NCORES)), **spmd_kwargs)
    out = np.empty((T, BS, C, HW), dtype=np.float32)
    shifts = np.arange(T, dtype=np.uint8)
    for k in range(NCORES):
        pk = np.asarray(res.results[k]["o_pk"], dtype=np.float32)  # [P, FREE]
        b = pk.astype(np.uint8).reshape(-1)                        # exact ints
        bits = np.unpackbits(b[:, None], axis=1, bitorder="little")[:, :T]
        o = bits.T.astype(np.float32).reshape(T, BSH, C, HW)
        out[:, k * BSH:(k + 1) * BSH] = o
    return out.reshape(T * BS, C, 32, 32), res


def kernel(x: np.ndarray) -> np.ndarray:
    out, _ = _run(x)
    return out


# revision 9
# speedup vs baseline: 1.6446x; 1.1371x over previous
"""LIF (leaky integrate-and-fire) spiking recurrence on 8 Trainium2 cores.

Full input x: [T*bs, C, H, W] = [256, 128, 32, 32] f32 with T=8, bs=32.
Recurrence over T only, elementwise elsewhere:
    u_t = TAU * u_{t-1} * (1 - (u_t-1 > VTH)) + x_t ;  o_t = (u_t > VTH)

Sharding: fully data-parallel over batch (bs=32 -> 4 per core), no collectives.

Each core views its per-timestep [4,128,32,32] slab as a [128, 4096] tile.
The kernel is HBM-bound on the 16 MiB input; the output (pure 0/1 spikes)
is bit-packed on device so stores are 1 MiB instead of 16 MiB:

  DVE : one fused custom op per step   u_t = x_t + TAU * u_{t-1} * (u_{t-1} <= VTH)
        (in-place over the x_t slab; bit-exact vs the f32 reference).
        At t=T-1 a second custom op emits the spike bit directly:
        o_7 = (x_7 + TAU * u_6 * (u_6 <= VTH)) > VTH, skipping u_7 and the
        scalar-engine pass on the critical tail.
  ACT : s_t = Sign(VTH - u_t) in bf16 for t < 7  (-1 = spike, +1 = not)
  PE  : psum += diag(-2^(t-1)) @ s_t  (t<7)  and  psum += diag(128) @ o_7
        => psum = sum_t 2^t*o_t - 63.5
  ACT : packed_bf16 = psum + 63.5     (integers 0..255, exact in bf16)
  DMA : store packed [128, 4096] bf16; host unpacks bits to the f32 output.

Loads are ramped with small DMAs at both ends: small first so compute can
start early, small last because a DMA only signals completion as a whole
and its final descriptors drain on a single SDMA engine (~27 GB/s) — a big
tail DMA would gate the last timestep chain for ~7 us.
"""

import numpy as np
import ml_dtypes

import concourse.tile as tile
from concourse import bacc, mybir
from concourse.bass_utils import run_bass_kernel_spmd

T = 8
BS = 32
C = 128
HW = 32 * 32
NCORES = 8
BSH = BS // NCORES          # 4 batch elements per core
P = 128                     # SBUF partitions
FREE = BSH * C * HW // P    # 4096 f32 per partition per timestep
VTH = 1.0
TAU = 0.5
F32 = mybir.dt.float32
BF16 = mybir.dt.bfloat16

_nc_cache = None
_ops_cache = None


def _register_ops():
    """Register two fused LIF custom DVE ops:
       LIF_STEP_ANT: out = Src1 + (Src0 * C0) * (Src0 <= One)    [u update]
       LIF_LAST_ANT: out = (Src1 + (Src0 * C0) * (Src0 <= One)) > One
    i.e. u_new = x + TAU*u*(u <= VTH), and the final-step spike bit."""
    global _ops_cache
    if _ops_cache is not None:
        return _ops_cache
    import concourse.dve_ops as dve_ops
    from concourse.dve_spec import Spec, Src0, Src1, C0, One, lower
    from concourse.dve_uop import DveOpSpec

    u_new = Src1 + (Src0 * C0) * (Src0 <= One)
    specs = {
        "LIF_STEP_ANT": Spec(
            body=u_new,
            reference=lambda in0, in1, c0, c1, c2: in1
            + (in0 * np.float32(c0)) * (in0 <= np.float32(1.0)),
        ),
        "LIF_LAST_ANT": Spec(
            body=u_new > One,
            reference=lambda in0, in1, c0, c1, c2: (
                in1 + (in0 * np.float32(c0)) * (in0 <= np.float32(1.0))
                > np.float32(1.0)
            ).astype(np.float32),
        ),
    }
    ops = {}
    by_name = {op.name: op for op in dve_ops.OPS}
    for name, spec in specs.items():
        if name in by_name:
            ops[name] = by_name[name]
            continue
        row = dve_ops._CUSTOM_DVE_ROW_BASE + len(dve_ops.OPS)
        uops_sha = {}
        for ver in ("v3", "v4"):
            try:
                s = DveOpSpec(
                    name=name, opcode=row, uops=lower(spec, ver=ver), rd1_en=True
                )
                uops_sha[ver] = s.sha(ver)
            except Exception:
                pass
        op = dve_ops.DveOp(name, spec, subdim=False, uops_sha=uops_sha)
        dve_ops.OPS.append(op)
        dve_ops._SUB_OPCODE_FOR_NAME[name] = row
        dve_ops.CUSTOM_DVE_SPECS[name] = spec
        ops[name] = op
    _ops_cache = ops
    return ops


# Column chunking per timestep: fine chunks at the pipeline head (t=0, so
# the scalar engine starts as soon as the first load lands) and toward the
# tail (t>=5, so the end-of-kernel chain drains with minimal latency).
_CHUNKS = {
    0: [(0, 1024), (1024, 2048), (2048, 3072), (3072, 4096)],
    5: [(0, 2048), (2048, 3072), (3072, 4096)],
    6: [(0, 1024), (1024, 2048), (2048, 3072), (3072, 4096)],
    7: [(0, 1024), (1024, 2048), (2048, 3072), (3072, 4096)],
}
_DEFAULT_CHUNKS = [(0, 2048), (2048, 4096)]

# Load schedule in units of 1024 columns (0.5 MiB). Small at both ends.
_LOADS = [(0, 1), (1, 2), (2, 4), (4, 8), (8, 12), (12, 16), (16, 20),
          (20, 24), (24, 26), (26, 28), (28, 30), (30, 31), (31, 32)]


def _build():
    ops = _register_ops()
    lif_step, lif_last = ops["LIF_STEP_ANT"], ops["LIF_LAST_ANT"]
    nc = bacc.Bacc("TRN2", target_bir_lowering=False, debug=False, num_devices=NCORES)
    x_d = nc.dram_tensor("x", [T, P, FREE], F32, kind="ExternalInput").ap()
    w_d = nc.dram_tensor("w", [P, T * 128], BF16, kind="ExternalInput").ap()
    o_d = nc.dram_tensor("o_pk", [P, FREE], BF16, kind="ExternalOutput").ap()

    with tile.TileContext(nc) as tc:
        with (
            tc.tile_pool(name="xa", bufs=1) as xa,
            tc.tile_pool(name="wp", bufs=1) as wp,
            tc.tile_pool(name="sp", bufs=3) as sp,
            tc.tile_pool(name="pk", bufs=1) as pkp,
            tc.tile_pool(name="cb", bufs=1) as cb,
            tc.tile_pool(name="ps", bufs=1, space="PSUM") as ps,
        ):
            # Whole 16 MiB per-core input resident in SBUF (128 KiB/partition);
            # u_t is computed in place over the x_t slab. Subtile dependency
            # tracking lets each compute chunk start once its load lands.
            xt = xa.tile([P, T * FREE], F32)
            xv = x_d.rearrange("t p f -> p t f")  # [128, T, FREE] HBM view

            wsb = wp.tile([P, T * 128], BF16)     # 8 stationary diag matrices
            nc.gpsimd.dma_start(out=wsb[:, :], in_=w_d)

            bias = cb.tile([P, 1], F32)
            nc.vector.memset(bias[:, :], 63.5)

            for a, b in _LOADS:
                t0, f0 = divmod(a * 1024, FREE)
                t1, f1 = divmod(b * 1024, FREE)
                if f0 == 0 and f1 == 0:
                    src = xv[:, t0:t1, :]
                else:
                    assert t1 == t0 or (t1 == t0 + 1 and f1 == 0)
                    src = xv[:, t0, f0:f1 if f1 else FREE]
                nc.sync.dma_start(out=xt[:, a * 1024:b * 1024], in_=src)

            psum = ps.tile([P, FREE], F32)        # packed-spike accumulator
            pk = pkp.tile([P, FREE], BF16)

            for t in range(T):
                last = t == T - 1
                s = sp.tile([P, FREE], BF16, name="s", tag="s")
                for a, b in _CHUNKS.get(t, _DEFAULT_CHUNKS):
                    xsl = xt[:, t * FREE + a:t * FREE + b]
                    if last:
                        # Fused u-update + threshold: o_7 directly in bf16.
                        nc.vector._custom_dve(
                            lif_last, out=s[:, a:b],
                            in0=xt[:, (t - 1) * FREE + a:(t - 1) * FREE + b],
                            in1=xsl, s0=TAU,
                        )
                    else:
                        if t > 0:
                            nc.vector._custom_dve(
                                lif_step, out=xsl,
                                in0=xt[:, (t - 1) * FREE + a:(t - 1) * FREE + b],
                                in1=xsl, s0=TAU,
                            )
                        # s = sign(VTH - u) in bf16: -1 = spike, +1 = not.
                        nc.scalar.activation(
                            s[:, a:b], xsl, mybir.ActivationFunctionType.Sign,
                            bias=VTH, scale=-1.0,
                        )
                    # psum += diag(-2^(t-1)) @ s_t (t<7); diag(128) @ o_7 (t=7)
                    for blk in range(a, b, 512):
                        nc.tensor.matmul(
                            psum[:, blk:blk + 512],
                            wsb[:, t * 128:(t + 1) * 128],
                            s[:, blk:blk + 512],
                            start=(t == 0),
                            stop=last,
                        )
                    if last:
                        # Convert psum -> packed bytes (0..255 ints, exact in
                        # bf16) and store, per chunk, right behind the PE.
                        nc.scalar.activation(
                            pk[:, a:b], psum[:, a:b],
                            mybir.ActivationFunctionType.Identity,
                            bias=bias[:, :], scale=1.0,
                        )
                        nc.gpsimd.dma_start(out=o_d[:, a:b], in_=pk[:, a:b])

    nc.compile()
    return nc


def _get_nc():
    global _nc_cache
    if _nc_cache is None:
        _nc_cache = _build()
    return _nc_cache


def _make_w():
    w = np.zeros((T, 128, 128), np.float32)
    for t in range(T - 1):
        np.fill_diagonal(w[t], -(2.0 ** (t - 1)))
    np.fill_diagonal(w[T - 1], 128.0)
    # SBUF layout: [partition k, t, m] -> [128, T*128]
    return np.ascontiguousarray(w.transpose(1, 0, 2)).reshape(P, T * 128).astype(
        ml_dtypes.bfloat16
    )


def _run(x: np.ndarray, **spmd_kwargs):
    nc = _get_nc()
    xr = np.ascontiguousarray(np.asarray(x, dtype=np.float32)).reshape(T, BS, C, HW)
    wl = _make_w()
    in_maps = [
        {
            "x": np.ascontiguousarray(xr[:, k * BSH:(k + 1) * BSH]).reshape(T, P, FREE),
            "w": wl,
        }
        for k in range(NCORES)
    ]
    res = run_bass_kernel_spmd(nc, in_maps, core_ids=list(range(NCORES)), **spmd_kwargs)
    out = np.empty((T, BS, C, HW), dtype=np.float32)
    for k in range(NCORES):
        pk = np.asarray(res.results[k]["o_pk"], dtype=np.float32)  # [P, FREE]
        b = pk.astype(np.uint8).reshape(-1)                        # exact ints
        bits = np.unpackbits(b[:, None], axis=1, bitorder="little")[:, :T]
        o = bits.T.astype(np.float32).reshape(T, BSH, C, HW)
        out[:, k * BSH:(k + 1) * BSH] = o
    return out.reshape(T * BS, C, 32, 32), res


def kernel(x: np.ndarray) -> np.ndarray:
    out, _ = _run(x)
    return out


# revision 12
# speedup vs baseline: 1.7558x; 1.0676x over previous
"""LIF (leaky integrate-and-fire) spiking recurrence on 8 Trainium2 cores.

Full input x: [T*bs, C, H, W] = [256, 128, 32, 32] f32 with T=8, bs=32.
Recurrence over T only, elementwise elsewhere:
    u_t = TAU * u_{t-1} * (1 - (u_t-1 > VTH)) + x_t ;  o_t = (u_t > VTH)

Sharding: fully data-parallel over batch (bs=32 -> 4 per core), no collectives.

Each core views its per-timestep [4,128,32,32] slab as a [128, 4096] tile.
The kernel is HBM-bound on the 16 MiB input; the output (pure 0/1 spikes)
is bit-packed on device so stores are 1 MiB instead of 16 MiB:

  DVE : one fused custom op per step   u_t = x_t + TAU * u_{t-1} * (u_{t-1} <= VTH)
        (in-place over the x_t slab; bit-exact vs the f32 reference).
        At t=T-1 a second custom op emits the spike bit directly:
        o_7 = (x_7 + TAU * u_6 * (u_6 <= VTH)) > VTH, skipping u_7 and the
        scalar-engine pass on the critical tail.
  ACT : s_t = Sign(VTH - u_t) in bf16 for t < 7  (-1 = spike, +1 = not)
  PE  : psum += diag(-2^(t-1)) @ s_t  (t<7)  and  psum += diag(128) @ o_7
        => psum = sum_t 2^t*o_t - 63.5
  ACT : packed_bf16 = psum + 63.5     (integers 0..255, exact in bf16)
  DMA : store packed [128, 4096] bf16; host unpacks bits to the f32 output.

Loads are ramped with small DMAs at both ends: small first so compute can
start early, small last because a DMA only signals completion as a whole
and its final descriptors drain on a single SDMA engine (~27 GB/s) — a big
tail DMA would gate the last timestep chain for ~7 us.
"""

import numpy as np
import ml_dtypes

import concourse.tile as tile
from concourse import bacc, mybir
from concourse.bass_utils import run_bass_kernel_spmd

T = 8
BS = 32
C = 128
HW = 32 * 32
NCORES = 8
BSH = BS // NCORES          # 4 batch elements per core
P = 128                     # SBUF partitions
FREE = BSH * C * HW // P    # 4096 f32 per partition per timestep
VTH = 1.0
TAU = 0.5
F32 = mybir.dt.float32
BF16 = mybir.dt.bfloat16

_nc_cache = None
_ops_cache = None


def _register_ops():
    """Register two fused LIF custom DVE ops:
       LIF_STEP_ANT: out = Src1 + (Src0 * C0) * (Src0 <= One)    [u update]
       LIF_LAST_ANT: out = (Src1 + (Src0 * C0) * (Src0 <= One)) > One
    i.e. u_new = x + TAU*u*(u <= VTH), and the final-step spike bit."""
    global _ops_cache
    if _ops_cache is not None:
        return _ops_cache
    import concourse.dve_ops as dve_ops
    from concourse.dve_spec import Spec, Src0, Src1, C0, One, lower
    from concourse.dve_uop import DveOpSpec

    u_new = Src1 + (Src0 * C0) * (Src0 <= One)
    specs = {
        "LIF_STEP_ANT": Spec(
            body=u_new,
            reference=lambda in0, in1, c0, c1, c2: in1
            + (in0 * np.float32(c0)) * (in0 <= np.float32(1.0)),
        ),
        "LIF_LAST_ANT": Spec(
            body=u_new > One,
            reference=lambda in0, in1, c0, c1, c2: (
                in1 + (in0 * np.float32(c0)) * (in0 <= np.float32(1.0))
                > np.float32(1.0)
            ).astype(np.float32),
        ),
    }
    ops = {}
    by_name = {op.name: op for op in dve_ops.OPS}
    for name, spec in specs.items():
        if name in by_name:
            ops[name] = by_name[name]
            continue
        row = dve_ops._CUSTOM_DVE_ROW_BASE + len(dve_ops.OPS)
        uops_sha = {}
        for ver in ("v3", "v4"):
            try:
                s = DveOpSpec(
                    name=name, opcode=row, uops=lower(spec, ver=ver), rd1_en=True
                )
                uops_sha[ver] = s.sha(ver)
            except Exception:
                pass
        op = dve_ops.DveOp(name, spec, subdim=False, uops_sha=uops_sha)
        dve_ops.OPS.append(op)
        dve_ops._SUB_OPCODE_FOR_NAME[name] = row
        dve_ops.CUSTOM_DVE_SPECS[name] = spec
        ops[name] = op
    _ops_cache = ops
    return ops


# Column chunking per timestep: fine chunks at the pipeline head (t<=1, so
# the DVE chain starts as soon as the first x_1 load lands) and toward the
# tail (t>=5, so the end-of-kernel chain drains with minimal latency).
_CHUNKS = {
    0: [(0, 1024), (1024, 2048), (2048, 3072), (3072, 4096)],
    1: [(0, 1024), (1024, 2048), (2048, 4096)],
    5: [(0, 2048), (2048, 3072), (3072, 4096)],
    6: [(0, 1024), (1024, 2048), (2048, 3072), (3072, 4096)],
    7: [(0, 1024), (1024, 2048), (2048, 3072), (3072, 4096)],
}
_DEFAULT_CHUNKS = [(0, 2048), (2048, 4096)]

# Load schedule in units of 1024 columns (0.5 MiB). Small at both ends:
# small first so the DVE recurrence starts early, small last because a DMA
# signals completion only as a whole.
_LOADS = [(0, 1), (1, 2), (2, 4), (4, 5), (5, 6), (6, 8), (8, 12), (12, 16),
          (16, 20), (20, 24), (24, 26), (26, 28), (28, 30), (30, 31), (31, 32)]


def _build():
    ops = _register_ops()
    lif_step, lif_last = ops["LIF_STEP_ANT"], ops["LIF_LAST_ANT"]
    nc = bacc.Bacc("TRN2", target_bir_lowering=False, debug=False, num_devices=NCORES)
    x_d = nc.dram_tensor("x", [T, P, FREE], F32, kind="ExternalInput").ap()
    w_d = nc.dram_tensor("w", [P, T * 128], BF16, kind="ExternalInput").ap()
    o_d = nc.dram_tensor("o_pk", [P, FREE], BF16, kind="ExternalOutput").ap()

    with tile.TileContext(nc) as tc:
        with (
            tc.tile_pool(name="xa", bufs=1) as xa,
            tc.tile_pool(name="wp", bufs=1) as wp,
            tc.tile_pool(name="sp", bufs=3) as sp,
            tc.tile_pool(name="pk", bufs=1) as pkp,
            tc.tile_pool(name="cb", bufs=1) as cb,
            tc.tile_pool(name="ps", bufs=1, space="PSUM") as ps,
        ):
            # Whole 16 MiB per-core input resident in SBUF (128 KiB/partition);
            # u_t is computed in place over the x_t slab. Subtile dependency
            # tracking lets each compute chunk start once its load lands.
            xt = xa.tile([P, T * FREE], F32)
            xv = x_d.rearrange("t p f -> p t f")  # [128, T, FREE] HBM view

            # All DMA goes through HWDGE rings (sync/scalar): keeping SWDGE
            # fully idle avoids its descriptor-ring SBUF port contention,
            # which slows SDMA engine 15 and makes it straggle ~7 us.
            wsb = wp.tile([P, T * 128], BF16)     # 8 stationary diag matrices
            nc.sync.dma_start(out=wsb[:, :], in_=w_d)

            bias = cb.tile([P, 1], F32)
            nc.vector.memset(bias[:, :], 63.5)

            for a, b in _LOADS:
                t0, f0 = divmod(a * 1024, FREE)
                t1, f1 = divmod(b * 1024, FREE)
                if f0 == 0 and f1 == 0:
                    src = xv[:, t0:t1, :]
                else:
                    assert t1 == t0 or (t1 == t0 + 1 and f1 == 0)
                    src = xv[:, t0, f0:f1 if f1 else FREE]
                nc.sync.dma_start(out=xt[:, a * 1024:b * 1024], in_=src)

            psum = ps.tile([P, FREE], F32)        # packed-spike accumulator
            pk = pkp.tile([P, FREE], BF16)

            for t in range(T):
                last = t == T - 1
                s = sp.tile([P, FREE], BF16, name="s", tag="s")
                for a, b in _CHUNKS.get(t, _DEFAULT_CHUNKS):
                    xsl = xt[:, t * FREE + a:t * FREE + b]
                    if last:
                        # Fused u-update + threshold: o_7 directly in bf16.
                        nc.vector._custom_dve(
                            lif_last, out=s[:, a:b],
                            in0=xt[:, (t - 1) * FREE + a:(t - 1) * FREE + b],
                            in1=xsl, s0=TAU,
                        )
                    else:
                        if t > 0:
                            nc.vector._custom_dve(
                                lif_step, out=xsl,
                                in0=xt[:, (t - 1) * FREE + a:(t - 1) * FREE + b],
                                in1=xsl, s0=TAU,
                            )
                        # s = sign(VTH - u) in bf16: -1 = spike, +1 = not.
                        nc.scalar.activation(
                            s[:, a:b], xsl, mybir.ActivationFunctionType.Sign,
                            bias=VTH, scale=-1.0,
                        )
                    # psum += diag(-2^(t-1)) @ s_t (t<7); diag(128) @ o_7 (t=7)
                    for blk in range(a, b, 512):
                        nc.tensor.matmul(
                            psum[:, blk:blk + 512],
                            wsb[:, t * 128:(t + 1) * 128],
                            s[:, blk:blk + 512],
                            start=(t == 0),
                            stop=last,
                        )
                    if last:
                        # Convert psum -> packed bytes (0..255 ints, exact in
                        # bf16) and store, per chunk, right behind the PE.
                        nc.scalar.activation(
                            pk[:, a:b], psum[:, a:b],
                            mybir.ActivationFunctionType.Identity,
                            bias=bias[:, :], scale=1.0,
                        )
                        nc.scalar.dma_start(out=o_d[:, a:b], in_=pk[:, a:b])

    nc.compile()
    return nc


def _get_nc():
    global _nc_cache
    if _nc_cache is None:
        _nc_cache = _build()
    return _nc_cache


def _make_w():
    w = np.zeros((T, 128, 128), np.float32)
    for t in range(T - 1):
        np.fill_diagonal(w[t], -(2.0 ** (t - 1)))
    np.fill_diagonal(w[T - 1], 128.0)
    # SBUF layout: [partition k, t, m] -> [128, T*128]
    return np.ascontiguousarray(w.transpose(1, 0, 2)).reshape(P, T * 128).astype(
        ml_dtypes.bfloat16
    )


def _run(x: np.ndarray, **spmd_kwargs):
    nc = _get_nc()
    xr = np.ascontiguousarray(np.asarray(x, dtype=np.float32)).reshape(T, BS, C, HW)
    wl = _make_w()
    in_maps = [
        {
            "x": np.ascontiguousarray(xr[:, k * BSH:(k + 1) * BSH]).reshape(T, P, FREE),
            "w": wl,
        }
        for k in range(NCORES)
    ]
    res = run_bass_kernel_spmd(nc, in_maps, core_ids=list(range(NCORES)), **spmd_kwargs)
    out = np.empty((T, BS, C, HW), dtype=np.float32)
    for k in range(NCORES):
        pk = np.asarray(res.results[k]["o_pk"], dtype=np.float32)  # [P, FREE]
        b = pk.astype(np.uint8).reshape(-1)                        # exact ints
        bits = np.unpackbits(b[:, None], axis=1, bitorder="little")[:, :T]
        o = bits.T.astype(np.float32).reshape(T, BSH, C, HW)
        out[:, k * BSH:(k + 1) * BSH] = o
    return out.reshape(T * BS, C, 32, 32), res


def kernel(x: np.ndarray) -> np.ndarray:
    out, _ = _run(x)
    return out
